# revision 2
# baseline (speedup 1.0000x reference)
"""Trainium2 Bass kernel for nn_AffineCouplingLayer (B=1048576, H=128).

  out[:, 0] = x[:, 0]
  out[:, 1] = x[:, 1] * exp(tanh(st0(z))) + st1(z),  z = x[:, 0]

The 2-layer MLP's output depends only on the scalar z, so st(z) is
piecewise-linear in z (kinks from both relu layers).  The host collapses
it into a single 128-unit relu layer (126 kinks chosen greedily from the
~207 exact kinks + linear + constant units, coefficients refit by
weighted least squares; max error ~7e-6 on the data range), and the
device evaluates:

  y_j = relu(z + V_j)           (K=1 fp32r broadcast matmuls, 4 row
                                 strips issued round-robin so they
                                 overlap in the PE array)
  st  = A^T y                   (fp32r matmuls, sliding-window weights
                                 accumulate 64 tiles into one PSUM bank:
                                 tile m lands on partitions 2m/2m+1)
  tail: tanh/exp on ACT, combine + interleave on DVE.

Pure data parallel across 8 NeuronCores (batch sharded, weights
replicated, no collectives).
"""
import numpy as np

import concourse.bass as bass
import concourse.tile as tile
import concourse.mybir as mybir
from concourse import bass_utils
from bass_rust import ScopedClock

FP = mybir.dt.float32
FR = mybir.dt.float32r
TILE = 512            # batch rows per tile (one fp32 PSUM bank)
TPSB = 128            # tiles per super-block
SBROWS = TILE * TPSB  # 65536
BLK = 64              # tiles accumulated per st PSUM block
CH = 32               # tiles per z-strip chunk
N_CORES = 8
N_SB = 2              # super-blocks per core
B_CORE = N_SB * SBROWS
B_FULL = N_CORES * B_CORE


# ---------------------------------------------------------------------------
# Tile framework shims for this walrus build (max 1 sync wait / instruction)
# ---------------------------------------------------------------------------
class FixedTileContext(tile.TileContext):
    def _drain_and_barrier(self, tick_clock, wait_clock):
        drain_inst = self.nc.sync.drain()
        wait_clock.add_sem_waits(
            drain_inst.ins, ScopedClock({None: tick_clock.global_clock})
        )
        si = drain_inst.ins.sync_info
        waits = list(si.on_wait) if si is not None else []
        if len(waits) > 1:
            drain_inst.ins.sync_info = mybir.SyncInfo(
                on_wait=waits[:1], on_update=list(si.on_update))
            for w in waits[1:]:
                nop = self.nc.sync.nop(hint="drain_wait_split", nofuse=True)
                nop.ins.sync_info = mybir.SyncInfo(on_wait=[w], on_update=[])
        self.nc.all_engine_barrier()
        assert self.sems is not None
        popped = self.nc._tile_sem_poison_stack.pop()
        assert popped is self._sem_poison
        self.nc.clear_and_free_semaphores(list(self.sems.allocated().values()))
        self.nc.all_engine_barrier()


def split_excess_waits(nc, cap=1):
    """Move excess on_wait entries onto same-engine NOPs inserted right
    before the instruction (same engine stream => they execute first)."""
    for fn in nc.m.functions:
        for blk in fn.blocks:
            il = blk.instructions
            i = 0
            while i < len(il):
                ins = il[i]
                si = getattr(ins, "sync_info", None)
                waits = list(si.on_wait) if si is not None else []
                if len(waits) > cap:
                    ins.sync_info = mybir.SyncInfo(
                        on_wait=waits[:cap], on_update=list(si.on_update))
                    for w in waits[cap:]:
                        nop = mybir.InstNoOp(name=f"wsplit-{nc.next_id()}")
                        nop.engine = ins.engine
                        nop.sync_info = mybir.SyncInfo(on_wait=[w], on_update=[])
                        nc.register_instruction(nop, overwrite=True)
                        il.insert(i, nop)
                        i += 1
                i += 1


# ---------------------------------------------------------------------------
# Host-side collapse: 2-layer MLP along scalar z -> 128-unit relu layer
# ---------------------------------------------------------------------------
def build_collapse(x, W1, b1, W2, b2, W3, b3, n_units=128):
    W1v = np.asarray(W1, np.float64).reshape(-1)
    b1v = np.asarray(b1, np.float64)
    W2v = np.asarray(W2, np.float64)
    b2v = np.asarray(b2, np.float64)
    W3v = np.asarray(W3, np.float64)
    b3v = np.asarray(b3, np.float64)
    z = np.asarray(x[:, 0], np.float64)
    lo = z.min() - 1e-3
    hi = z.max() + 1e-3

    def st_exact(zv):
        h1 = np.maximum(np.outer(zv, W1v) + b1v, 0)
        h2 = np.maximum(h1 @ W2v + b2v, 0)
        return h2 @ W3v + b3v

    with np.errstate(divide="ignore", invalid="ignore"):
        k1 = -b1v / W1v
    k1 = k1[np.isfinite(k1)]
    k1_in = k1[(k1 > lo) & (k1 < hi)]
    bnds = np.sort(np.concatenate([k1, [lo - 50, hi + 50]]))
    gb = np.maximum(np.outer(bnds, W1v) + b1v, 0) @ W2v + b2v
    cross = []
    for j in range(gb.shape[1]):
        gj = gb[:, j]
        idx = np.nonzero((gj[:-1] < 0) != (gj[1:] < 0))[0]
        for i in idx:
            t = gj[i] / (gj[i] - gj[i + 1])
            cz = bnds[i] + t * (bnds[i + 1] - bnds[i])
            if lo < cz < hi:
                cross.append(cz)
    kinks = np.sort(np.concatenate([k1_in, np.array(cross)]))

    pts = np.concatenate([[lo], kinks, [hi]])
    st_l = st_exact(pts[:-1])
    st_r = st_exact(pts[1:])
    seg_slope = (st_r - st_l) / (pts[1:] - pts[:-1])[:, None]
    a = seg_slope[1:] - seg_slope[:-1]

    n_kinks = n_units - 2
    r = kinks.copy()
    aa = a.copy()
    W0 = 14.0  # st0 error amplification bound through ztr*exp(tanh(.))
    while len(r) > n_kinks:
        d = np.minimum(np.diff(r, prepend=lo), np.diff(r, append=hi))
        imp = np.maximum(W0 * np.abs(aa[:, 0]), np.abs(aa[:, 1])) * d
        i = int(np.argmin(imp))
        dl = r[i] - (r[i - 1] if i > 0 else lo)
        dr = (r[i + 1] if i < len(r) - 1 else hi) - r[i]
        j = i - 1 if (dl <= dr and i > 0) else (i + 1 if i < len(r) - 1 else i - 1)
        aa[j] = aa[j] + aa[i]
        r = np.delete(r, i)
        aa = np.delete(aa, i, axis=0)
    if len(r) < n_kinks:
        r = np.concatenate([r, hi + 1.0 + np.arange(n_kinks - len(r))])
    r = np.sort(r)

    grid = np.unique(np.concatenate([np.linspace(lo, hi, 8000), kinks, r]))
    Phi = np.concatenate(
        [np.ones((len(grid), 1)), (grid - lo)[:, None],
         np.maximum(grid[:, None] - r[None, :], 0)], axis=1)
    tv = st_exact(grid)
    w = np.ones(len(grid))
    coef = None
    for _ in range(4):
        cw = Phi * w[:, None]
        c0, *_ = np.linalg.lstsq(cw, tv[:, 0] * w, rcond=None)
        c1, *_ = np.linalg.lstsq(cw, tv[:, 1] * w, rcond=None)
        coef = np.stack([c0, c1], axis=1)
        err = Phi @ coef - tv
        m = np.maximum(W0 * np.abs(err[:, 0]), np.abs(err[:, 1]))
        mx = m.max() + 1e-30
        w = (0.1 * mx + m) / mx
        w /= w.mean()

    U = np.ones(n_units, np.float64)
    V = np.empty(n_units, np.float64)
    V[:n_kinks] = -r
    V[n_kinks] = -(lo - 1.0)
    U[n_kinks + 1] = 0.0
    V[n_kinks + 1] = 1.0
    A = np.zeros((n_units, 2), np.float64)
    A[:n_kinks] = coef[2:]
    A[n_kinks] = coef[1]
    A[n_kinks + 1] = coef[0] - coef[1]
    return U.astype(np.float32), V.astype(np.float32), A.astype(np.float32)


def round_fp32r(x):
    """Round fp32 to the fp32r grid (1-8-11, low 12 mantissa bits zero)."""
    b = np.ascontiguousarray(np.asarray(x, np.float32)).view(np.uint32)
    add = np.uint32(1 << 11)
    tie = ((b >> 12) & 1).astype(np.uint32)
    out = (b + add - np.uint32(1) + tie) & np.uint32(0xFFFFF000)
    return out.view(np.float32)


# ---------------------------------------------------------------------------
# Device kernel
# ---------------------------------------------------------------------------
def build_device_kernel(n_sb=N_SB, evac_act_ratio=0.54, warmup=12, repeat=1):
    B = n_sb * SBROWS
    nc = bass.Bass()
    x_d = nc.dram_tensor("x", (B, 2), FP, kind="ExternalInput")
    urep_d = nc.dram_tensor("urep", (128, 128), FP, kind="ExternalInput")
    vb_d = nc.dram_tensor("vb", (128, 1), FP, kind="ExternalInput")
    apads_d = nc.dram_tensor("apads", (128, 254), FP, kind="ExternalInput")
    out_d = nc.dram_tensor("out", (B, 2), FP, kind="ExternalOutput")

    Relu = mybir.ActivationFunctionType.Relu
    Tanh = mybir.ActivationFunctionType.Tanh
    Exp = mybir.ActivationFunctionType.Exp
    add = mybir.AluOpType.add
    amax = mybir.AluOpType.max

    with FixedTileContext(nc) as tc:
        dma_eng = nc.sync
        with (
            tc.tile_pool(name="consts", bufs=1) as cpool,
            tc.tile_pool(name="xin", bufs=2) as xpool,
            tc.tile_pool(name="zext", bufs=2) as zpool,
            tc.tile_pool(name="zstrip", bufs=3) as spool,
            tc.tile_pool(name="ysb", bufs=8) as ypool,
            tc.tile_pool(name="stal", bufs=3) as alpool,
            tc.tile_pool(name="tail", bufs=3) as tpool,
            tc.tile_pool(name="outb", bufs=2) as opool,
            tc.tile_pool(name="psy", bufs=6, space=bass.MemorySpace.PSUM) as psy,
            tc.tile_pool(name="psst", bufs=2, space=bass.MemorySpace.PSUM) as psst,
        ):
            urep = cpool.tile([128, 128], FR)
            dma_eng.dma_start(urep[:], urep_d[:].bitcast(FR))
            vb = cpool.tile([128, 1], FP)
            dma_eng.dma_start(vb[:], vb_d[:])
            apads = cpool.tile([128, 254], FR)
            dma_eng.dma_start(apads[:], apads_d[:].bitcast(FR))

            # HAM warmup while input DMAs are in flight; the first real
            # block's start=True matmul overwrites this PSUM slot.
            warm_ps = psst.tile([128, TILE], FP, tag="st_ps")
            for _ in range(warmup):
                nc.tensor.matmul(warm_ps[:, 0:128], urep[:], urep[:],
                                 start=True, stop=True)

            def stage_in(s, pieces):
                x_sb = xpool.tile([128, 2 * TILE], FP, name="x_sb")
                z_q = zpool.tile([128, TILE], FR, tag="z", name="z_q")
                ztr_q = zpool.tile([128, TILE], FP, tag="ztr", name="ztr_q")
                np_ = 128 // pieces
                for p in range(pieces):
                    rows = slice(np_ * p, np_ * (p + 1))
                    xs = x_d[s * SBROWS + np_ * p * TILE:
                             s * SBROWS + np_ * (p + 1) * TILE, :].rearrange(
                        "(q r) c -> q (r c)", q=np_)
                    dma_eng.dma_start(x_sb[rows, :], xs)
                    nc.vector.tensor_copy(z_q[rows, :], x_sb[rows, 0::2])
                    nc.vector.tensor_copy(ztr_q[rows, :], x_sb[rows, 1::2])
                return x_sb, z_q, ztr_q

            def scatter_chunk(z_q, c):
                # strip k <- tiles [32c+8k, 32c+8k+8), each strip one
                # 32-aligned partition (matmul operand requirement)
                zs = spool.tile([128, TILE * CH // 4], FR, name="zs")
                for k in range(4):
                    dma_eng.dma_start(
                        zs[32 * k:32 * k + 1, :],
                        z_q[CH * c + 8 * k:CH * c + 8 * (k + 1), :])
                return zs

            import contextlib
            loop_ctx = (tc.For_i(0, repeat, 1) if repeat > 1
                        else contextlib.nullcontext())
            with loop_ctx:
                staged = {0: stage_in(0, 4)}
                zstrips = {(0, 0): scatter_chunk(staged[0][1], 0)}
                act_quota = 0.0
                for s in range(n_sb):
                    x_sb, z_q, ztr_q = staged[s]
                    for blk in range(TPSB // BLK):
                        st_ps = psst.tile([128, TILE], FP, tag="st_ps",
                                          name="st_ps")
                        r0 = blk * BLK
                        rs = slice(r0, r0 + BLK)
                        out_blk = opool.tile([128, 2 * TILE], FP,
                                             name="out_blk")
                        nc.vector.tensor_copy(
                            out_blk[rs, 0::2], x_sb[rs, 0::2])

                        first = True
                        ngroups = 0
                        for c in range(2 * blk, 2 * blk + 2):
                            if (s, c) not in zstrips:
                                zstrips[(s, c)] = scatter_chunk(z_q, c)
                            for i in range(8):
                                # 4 tiles on 4 different row strips: their
                                # K=1 matmuls overlap in the PE array
                                tiles = [CH * c + 8 * k + i for k in range(4)]
                                y_list = []
                                for k, t in enumerate(tiles):
                                    y_ps = psy.tile([128, TILE], FP,
                                                    tag="y_ps", name="y_ps")
                                    nc.tensor.matmul(
                                        y_ps[:],
                                        urep[32 * k:32 * k + 1, :],
                                        zstrips[(s, c)][
                                            32 * k:32 * k + 1,
                                            TILE * i:TILE * (i + 1)],
                                        start=True, stop=True,
                                        tile_position=(32 * k, 0),
                                    )
                                    y_list.append(y_ps)
                                y_sbs = []
                                for y_ps in y_list:
                                    y_sb = ypool.tile([128, TILE], FR,
                                                      tag="y_sb", name="y_sb")
                                    act_quota += evac_act_ratio
                                    if act_quota >= 1.0:
                                        act_quota -= 1.0
                                        nc.scalar.activation(
                                            y_sb[:], y_ps[:], Relu,
                                            bias=vb[:])
                                    else:
                                        nc.vector.tensor_scalar(
                                            y_sb[:], y_ps[:], vb[:], 0.0,
                                            op0=add, op1=amax)
                                    y_sbs.append(y_sb)
                                last_group = (c == 2 * blk + 1) and (i == 7)
                                for k, t in enumerate(tiles):
                                    m = t - blk * BLK
                                    nc.tensor.matmul(
                                        st_ps[:],
                                        apads[:, 126 - 2 * m:254 - 2 * m],
                                        y_sbs[k][:],
                                        start=first,
                                        stop=last_group and (k == 3),
                                    )
                                    first = False
                                ngroups += 1
                                if (i == 3 and c + 1 < 4
                                        and (s, c + 1) not in zstrips):
                                    zstrips[(s, c + 1)] = scatter_chunk(
                                        z_q, c + 1)
                                if s + 1 < n_sb and blk == 0 and ngroups == 3:
                                    staged[s + 1] = stage_in(s + 1, 1)
                                if s + 1 < n_sb and blk == 1 and ngroups == 10:
                                    zstrips[(s + 1, 0)] = scatter_chunk(
                                        staged[s + 1][1], 0)

                        # block epilogue: st -> (s|t) rows -> tanh/exp ->
                        # combine -> interleaved output
                        stb = tpool.tile([128, TILE], FP, tag="stb",
                                         name="stb")
                        nc.vector.tensor_copy(stb[:], st_ps[:])
                        stal = alpool.tile([128, 2 * TILE], FP, tag="stal",
                                           name="stal")
                        dma_eng.dma_start(stal[rs, :], stb[:])
                        s2 = tpool.tile([128, TILE], FP, tag="s2", name="s2")
                        nc.scalar.activation(s2[rs, :], stal[rs, 0:TILE], Tanh)
                        ex = tpool.tile([128, TILE], FP, tag="ex", name="ex")
                        nc.scalar.activation(ex[rs, :], s2[rs, :], Exp)
                        prod = tpool.tile([128, TILE], FP, tag="prod",
                                          name="prod")
                        nc.vector.tensor_mul(
                            prod[rs, :], ex[rs, :], ztr_q[rs, :])
                        nc.vector.tensor_add(
                            out_blk[rs, 1::2], prod[rs, :], stal[rs, TILE:])
                        orows = out_d[s * SBROWS + r0 * TILE:
                                      s * SBROWS + (r0 + BLK) * TILE, :]
                        dma_eng.dma_start(
                            orows.rearrange("(q r) c -> q (r c)", q=BLK),
                            out_blk[rs, :])
    split_excess_waits(nc)
    nc.finalize()
    return nc


def make_in_maps(x_full, U, V, A, n_cores=N_CORES, n_sb=N_SB):
    B = n_sb * SBROWS
    U = round_fp32r(U)
    A = round_fp32r(A)
    urep = np.tile(U.reshape(1, 128), (128, 1)).astype(np.float32)
    vbm = np.ascontiguousarray(V.reshape(128, 1).astype(np.float32))
    apads = np.zeros((128, 254), np.float32)
    apads[:, 126] = A[:, 0]
    apads[:, 127] = A[:, 1]
    maps = []
    for c in range(n_cores):
        maps.append({
            "x": np.ascontiguousarray(x_full[c * B:(c + 1) * B]),
            "urep": urep, "vb": vbm, "apads": apads,
        })
    return maps


_NC_CACHE = {}


def _get_nc(repeat=1):
    if repeat not in _NC_CACHE:
        _NC_CACHE[repeat] = build_device_kernel(repeat=repeat)
    return _NC_CACHE[repeat]


def kernel(x, W1, b1, W2, b2, W3, b3):
    x = np.ascontiguousarray(np.asarray(x, np.float32))
    assert x.shape == (B_FULL, 2), x.shape
    U, V, A = build_collapse(x, W1, b1, W2, b2, W3, b3)
    nc = _get_nc()
    maps = make_in_maps(x, U, V, A)
    res = bass_utils.run_bass_kernel_spmd(
        nc, maps, core_ids=list(range(N_CORES)))
    out = np.concatenate([res.results[c]["out"] for c in range(N_CORES)],
                         axis=0)
    return out
